# revision 17
# baseline (speedup 1.0000x reference)
"""Trainium2 Bass kernel for nn_Block_89172110999857 (dense transformer decode block).

Sharding: tensor-parallel over 8 NeuronCores.
  - Attention: 2 heads per core (H=16). KV cache for those heads streamed in bf16.
  - attn out-proj: row-sharded per head -> partial [B, C], AllReduce across cores.
  - MLP: Megatron col/row sharding (w_fc columns, w_mlp_proj rows); partial outputs
    summed on the host.
Numerics:
  - QKV projection in fp32 (k_new / v_new are direct outputs).
  - Attention scores / AV and the MLP in bf16 inputs with fp32 PSUM accumulation.
  - Softmax: unnormalized exp (scores are O(1); no max subtraction needed), additive
    host-built causal mask (-1e30), denominator via fused accum_out on the exp, final
    normalization folded into per-head projection rows (tensor_scalar by 1/den).
"""

import math

import numpy as np
import ml_dtypes

import concourse.bacc as bacc
import concourse.mybir as mybir
import concourse.tile as tile
from concourse.bass import ts
from concourse.bass_utils import run_bass_kernel_spmd
from concourse.masks import make_identity

B, T, H, HD = 32, 2048, 16, 128
C = H * HD  # 2048
N_CORES = 8
HPC = H // N_CORES  # 2 heads per core
NPAIR = B * HPC  # 64 (b, h) pairs per core
NCHUNK = T // 128  # 16 t-chunks
G = 4  # pairs per KV DMA group
EPS = 1e-5
NEG = -1e30

F32 = mybir.dt.float32
BF16 = mybir.dt.bfloat16
BF = ml_dtypes.bfloat16
AF = mybir.ActivationFunctionType
ALU = mybir.AluOpType

_CACHE: dict = {}


def _layernorm(nc, pool, x_sb, out_tile, eps_sb, name):
    """out_tile = (x - mean(x)) * rsqrt(var(x) + eps), rows = partitions."""
    xg = x_sb.rearrange("b (n f) -> b n f", f=512)
    stats = pool.tile([B, 4, 6], F32, name=f"{name}_stats", tag="ln_stats")
    for i in range(4):
        nc.vector.bn_stats(out=stats[:, i, :], in_=xg[:, i, :])
    mv = pool.tile([B, 2], F32, name=f"{name}_mv", tag="ln_mv")
    nc.vector.bn_aggr(out=mv, in_=stats)
    std = pool.tile([B, 1], F32, name=f"{name}_std", tag="ln_std")
    nc.scalar.activation(out=std, in_=mv[:, 1:2], func=AF.Sqrt, bias=eps_sb, scale=1.0)
    rstd = pool.tile([B, 1], F32, name=f"{name}_rstd", tag="ln_rstd")
    nc.vector.reciprocal(out=rstd, in_=std)
    nc.vector.tensor_scalar(
        out=out_tile, in0=x_sb, scalar1=mv[:, 0:1], scalar2=rstd,
        op0=ALU.subtract, op1=ALU.mult,
    )


def _transpose_rows(nc, trps, pool, src_sb, nchunks, out_dtype, identity, name):
    """src_sb [32, nchunks*128] -> dst [128, nchunks, 32] (cast to out_dtype)."""
    dst = pool.tile([128, nchunks, 32], out_dtype, name=name)
    for i in range(nchunks):
        trp = trps.tile([128, 32], src_sb.dtype, name=f"{name}_trp{i}", tag="trp")
        nc.tensor.transpose(out=trp, in_=src_sb[:, ts(i, 128)], identity=identity)
        nc.vector.tensor_copy(out=dst[:, i, :], in_=trp)
    return dst


def _build(flags):
    import os
    sim_1core = os.environ.get("KERNEL_SIM_1CORE") == "1"
    has_bq, has_bfc, has_bproj = flags
    nc = bacc.Bacc(
        "TRN2", target_bir_lowering=False, debug=False,
        num_devices=1 if sim_1core else N_CORES,
    )

    x_d = nc.dram_tensor("x", [B, C], F32, kind="ExternalInput").ap()
    kdev_d = nc.dram_tensor("kdev", [NPAIR, HD, T], BF16, kind="ExternalInput").ap()
    vdev_d = nc.dram_tensor("vdev", [NPAIR, 128, NCHUNK, HD], BF16, kind="ExternalInput").ap()
    mask_d = nc.dram_tensor("maskadd", [128, B, NCHUNK], F32, kind="ExternalInput").ap()
    wqkv_d = nc.dram_tensor("wqkv", [128, 16, 3 * HPC * HD], F32, kind="ExternalInput").ap()
    wproj_d = nc.dram_tensor("wproj", [128, HPC, C], BF16, kind="ExternalInput").ap()
    wfc_d = nc.dram_tensor("wfc", [128, 16, 1024], BF16, kind="ExternalInput").ap()
    wmlp_d = nc.dram_tensor("wmlp", [128, 8, C], BF16, kind="ExternalInput").ap()
    if has_bq:
        bq_d = nc.dram_tensor("bqT", [HD, HPC], F32, kind="ExternalInput").ap()
    if has_bfc:
        bfc_d = nc.dram_tensor("bfc", [1024], F32, kind="ExternalInput").ap()
    if has_bproj:
        bproj_d = nc.dram_tensor("bproj", [C], F32, kind="ExternalInput").ap()

    y_d = nc.dram_tensor("y_mlp", [B, C], F32, kind="ExternalOutput").ap()
    x2_d = nc.dram_tensor("x2", [B, C], F32, kind="ExternalOutput").ap()
    knew_d = nc.dram_tensor("knew", [B, HPC * HD], F32, kind="ExternalOutput").ap()
    vnew_d = nc.dram_tensor("vnew", [B, HPC * HD], F32, kind="ExternalOutput").ap()

    QN = HPC * HD  # 256: per-core q/k/v width

    with tile.TileContext(nc) as tc:
        from contextlib import ExitStack

        with ExitStack() as ctx:
            const = ctx.enter_context(tc.tile_pool(name="const", bufs=1))
            work = ctx.enter_context(tc.tile_pool(name="work", bufs=1))
            wstream = ctx.enter_context(tc.tile_pool(name="wstream", bufs=2))
            wpool = ctx.enter_context(tc.tile_pool(name="wpool", bufs=1))
            kvp = ctx.enter_context(tc.tile_pool(name="kvp", bufs=2))
            dram = ctx.enter_context(tc.tile_pool(name="dram", bufs=1, space="DRAM"))

            # ---- constants ----
            id_f = const.tile([32, 32], F32)
            make_identity(nc, id_f)
            id_b = const.tile([32, 32], BF16)
            make_identity(nc, id_b)
            eps_sb = const.tile([B, 1], F32)
            nc.vector.memset(eps_sb, EPS)
            ones_sb = const.tile([128, 1], F32)
            nc.vector.memset(ones_sb, 1.0)

            mask_sb = const.tile([128, B, NCHUNK], F32)
            nc.sync.dma_start(out=mask_sb, in_=mask_d)
            wproj_sb = wpool.tile([128, HPC, C], BF16)
            nc.sync.dma_start(out=wproj_sb, in_=wproj_d)

            x_sb = const.tile([B, C], F32)
            nc.sync.dma_start(out=x_sb, in_=x_d)

            # ---- LN1 + QKV (fp32) ----
            h_sb = work.tile([B, C], F32, name="h_sb")
            _layernorm(nc, work, x_sb, h_sb, eps_sb, "ln1")

            qT = work.tile([128, NPAIR], BF16, name="qT")
            with tc.tile_pool(name="trps1", bufs=2, space="PSUM") as trps, \
                 tc.tile_pool(name="qkvps", bufs=1, space="PSUM") as qkvps:
                hT = _transpose_rows(nc, trps, work, h_sb, 16, F32, id_f, "hT")

                q_ps = qkvps.tile([B, QN], F32, name="q_ps")
                k_ps = qkvps.tile([B, QN], F32, name="k_ps")
                v_ps = qkvps.tile([B, QN], F32, name="v_ps")
                for kc in range(16):
                    wqc = wstream.tile([128, 3 * QN], F32, name="wqc", tag="wqc")
                    nc.sync.dma_start(out=wqc, in_=wqkv_d[:, kc, :])
                    for blk, (n0, n1) in zip(
                        (q_ps, k_ps, v_ps), ((0, QN), (QN, 2 * QN), (2 * QN, 3 * QN))
                    ):
                        nc.tensor.matmul(
                            out=blk, lhsT=hT[:, kc, :], rhs=wqc[:, n0:n1],
                            start=(kc == 0), stop=(kc == 15),
                        )

                k_sb = work.tile([B, QN], F32, name="k_sb")
                nc.vector.tensor_copy(out=k_sb, in_=k_ps)
                nc.sync.dma_start(out=knew_d, in_=k_sb)
                v_sb = work.tile([B, QN], F32, name="v_sb")
                nc.vector.tensor_copy(out=v_sb, in_=v_ps)
                nc.sync.dma_start(out=vnew_d, in_=v_sb)

                # qT [128 d, (h b)] bf16 (1/sqrt(HD) folded into wq on host)
                q_sb = work.tile([B, QN], F32, name="q_sb")
                nc.vector.tensor_copy(out=q_sb, in_=q_ps)
                if has_bq:
                    bq_sb = const.tile([HD, HPC], F32)
                    nc.sync.dma_start(out=bq_sb, in_=bq_d)
                for h in range(HPC):
                    qtp = trps.tile([128, 32], F32, name=f"qtp{h}", tag="trp")
                    nc.tensor.transpose(out=qtp, in_=q_sb[:, ts(h, HD)], identity=id_f)
                    if has_bq:
                        nc.vector.tensor_scalar(
                            out=qT[:, ts(h, 32)], in0=qtp, scalar1=bq_sb[:, h : h + 1],
                            scalar2=None, op0=ALU.add,
                        )
                    else:
                        nc.vector.tensor_copy(out=qT[:, ts(h, 32)], in_=qtp)

            # ---- attention over 64 (b, h) pairs ----
            den_all = work.tile([128, NPAIR], F32, name="den_all")
            den_inv = work.tile([B, HPC], F32, name="den_inv")
            attnT_sb = work.tile([128, NPAIR], BF16, name="attnT_sb")

            with tc.tile_pool(name="scps", bufs=2, space="PSUM") as scps, \
                 tc.tile_pool(name="atps", bufs=1, space="PSUM") as atps:
                attnT_ps = [
                    atps.tile([128, B], F32, name=f"attnT{h}") for h in range(HPC)
                ]

                for j0 in range(0, NPAIR, G):
                    k_t = kvp.tile([128, G, T], BF16, name="k_t")
                    nc.sync.dma_start(
                        out=k_t, in_=kdev_d[j0 : j0 + G].rearrange("j p t -> p j t")
                    )
                    v_t = kvp.tile([128, G, NCHUNK, HD], BF16, name="v_t")
                    nc.sync.dma_start(
                        out=v_t, in_=vdev_d[j0 : j0 + G].rearrange("j p c d -> p j c d")
                    )
                    # software-pipelined trace order within the group: all
                    # scores blocks first, then softmax, then all AV blocks —
                    # keeps the PE stream free of per-pair DVE/ACT stalls.
                    scs = []
                    for jj in range(G):
                        j = j0 + jj
                        sc = scps.tile(
                            [128, NCHUNK], F32, name="sc", tag="sc", bufs=G + 1
                        )
                        scs.append(sc)
                        for c in range(NCHUNK):
                            nc.tensor.matmul(
                                out=sc[:, c : c + 1], lhsT=k_t[:, jj, ts(c, 128)],
                                rhs=qT[:, j : j + 1], start=True, stop=True,
                                skip_group_check=True,
                            )
                    prs = []
                    for jj in range(G):
                        j = j0 + jj
                        b = j % B
                        sm = work.tile(
                            [128, NCHUNK], F32, name="sm", tag="sm", bufs=G + 1
                        )
                        nc.vector.tensor_add(out=sm, in0=scs[jj], in1=mask_sb[:, b, :])
                        pr = work.tile(
                            [128, NCHUNK], BF16, name="pr", tag="pr", bufs=G + 1
                        )
                        prs.append(pr)
                        nc.scalar.activation(
                            out=pr, in_=sm, func=AF.Exp, accum_out=den_all[:, j : j + 1]
                        )
                    for jj in range(G):
                        j = j0 + jj
                        b, h = j % B, j // B
                        for c in range(NCHUNK):
                            nc.tensor.matmul(
                                out=attnT_ps[h][:, b : b + 1], lhsT=v_t[:, jj, c, :],
                                rhs=prs[jj][:, c : c + 1],
                                start=(c == 0), stop=(c == NCHUNK - 1),
                                skip_group_check=True,
                            )

                # denominators -> 1/den per head [32, 1]
                for h in range(HPC):
                    den_ps = scps.tile([B, 1], F32, name=f"den_ps{h}", tag="den_ps", bufs=1)
                    nc.tensor.matmul(
                        out=den_ps, lhsT=den_all[:, ts(h, 32)], rhs=ones_sb,
                        start=True, stop=True,
                    )
                    nc.vector.reciprocal(out=den_inv[:, h : h + 1], in_=den_ps)

                for h in range(HPC):
                    nc.vector.tensor_copy(out=attnT_sb[:, ts(h, 32)], in_=attnT_ps[h])

            # per-head out-proj (contraction d=128), normalize rows by 1/den
            attn_par = work.tile([B, C], F32, name="attn_par")
            with tc.tile_pool(name="projps", bufs=1, space="PSUM") as projps:
                proj_ps = [
                    projps.tile([B, C], F32, name=f"proj_ps{h}") for h in range(HPC)
                ]
                for h in range(HPC):
                    for nb in range(C // 512):
                        nc.tensor.matmul(
                            out=proj_ps[h][:, ts(nb, 512)], lhsT=attnT_sb[:, ts(h, 32)],
                            rhs=wproj_sb[:, h, ts(nb, 512)], start=True, stop=True,
                        )
                pn0 = work.tile([B, C], F32, name="pn0")
                nc.vector.tensor_scalar_mul(
                    out=pn0, in0=proj_ps[0], scalar1=den_inv[:, 0:1]
                )
                pn1 = work.tile([B, C], F32, name="pn1")
                nc.vector.tensor_scalar_mul(
                    out=pn1, in0=proj_ps[1], scalar1=den_inv[:, 1:2]
                )
                nc.vector.tensor_add(out=attn_par, in0=pn0, in1=pn1)

            # ---- AllReduce of the attention partial ----
            ar_in = dram.tile([B, C], F32, name="ar_in")
            ar_out = dram.tile([B, C], F32, name="ar_out", addr_space="Shared")
            nc.sync.dma_start(out=ar_in, in_=attn_par)
            if sim_1core:
                nc.sync.dma_start(out=ar_out, in_=ar_in)
            else:
                nc.gpsimd.collective_compute(
                    "AllReduce", ALU.add,
                    replica_groups=[list(range(N_CORES))],
                    ins=[ar_in.opt()], outs=[ar_out.opt()],
                )
            ar_sb = work.tile([B, C], F32, name="ar_sb")
            nc.sync.dma_start(out=ar_sb, in_=ar_out)

            x2_sb = work.tile([B, C], F32, name="x2_sb")
            if has_bproj:
                bproj_sb = const.tile([B, C], F32)
                import concourse.bass as bass_mod
                bproj_bcast = bass_mod.AP(
                    tensor=bproj_d.tensor, offset=0, ap=[[0, B], [1, C]]
                )
                nc.sync.dma_start(out=bproj_sb, in_=bproj_bcast)
                tmpb = work.tile([B, C], F32, name="tmpb")
                nc.vector.tensor_add(out=tmpb, in0=ar_sb, in1=bproj_sb)
                nc.vector.tensor_add(out=x2_sb, in0=x_sb, in1=tmpb)
            else:
                nc.vector.tensor_add(out=x2_sb, in0=x_sb, in1=ar_sb)
            nc.sync.dma_start(out=x2_d, in_=x2_sb)

            # ---- LN2 + MLP (bf16) ----
            h2_bf = work.tile([B, C], BF16, name="h2_bf")
            _layernorm(nc, work, x2_sb, h2_bf, eps_sb, "ln2")

            with tc.tile_pool(name="trps2", bufs=2, space="PSUM") as trps2, \
                 tc.tile_pool(name="mlpps", bufs=1, space="PSUM") as mlpps:
                h2T = _transpose_rows(nc, trps2, work, h2_bf, 16, BF16, id_b, "h2T")

                fc_ps = mlpps.tile([B, 1024], F32, name="fc_ps")
                for kc in range(16):
                    wfcc = wstream.tile([128, 1024], BF16, name="wfcc", tag="wfcc")
                    nc.sync.dma_start(out=wfcc, in_=wfc_d[:, kc, :])
                    for nb in range(2):
                        nc.tensor.matmul(
                            out=fc_ps[:, ts(nb, 512)], lhsT=h2T[:, kc, :],
                            rhs=wfcc[:, ts(nb, 512)],
                            start=(kc == 0), stop=(kc == 15), skip_group_check=True,
                        )
                h2g = work.tile([B, 1024], BF16, name="h2g")
                if has_bfc:
                    bfc_sb = const.tile([B, 1024], F32)
                    import concourse.bass as bass_mod
                    bfc_bcast = bass_mod.AP(
                        tensor=bfc_d.tensor, offset=0, ap=[[0, B], [1, 1024]]
                    )
                    nc.sync.dma_start(out=bfc_sb, in_=bfc_bcast)
                    fcb = work.tile([B, 1024], F32, name="fcb")
                    nc.vector.tensor_add(out=fcb, in0=fc_ps, in1=bfc_sb)
                    nc.scalar.activation(out=h2g, in_=fcb, func=AF.Gelu_apprx_tanh)
                else:
                    nc.scalar.activation(out=h2g, in_=fc_ps, func=AF.Gelu_apprx_tanh)

                h2gT = _transpose_rows(nc, trps2, work, h2g, 8, BF16, id_b, "h2gT")
                mlp_ps = mlpps.tile([B, C], F32, name="mlp_ps")
                for kc in range(8):
                    wmlpc = wstream.tile([128, C], BF16, name="wmlpc", tag="wmlpc")
                    nc.sync.dma_start(out=wmlpc, in_=wmlp_d[:, kc, :])
                    for nb in range(4):
                        nc.tensor.matmul(
                            out=mlp_ps[:, ts(nb, 512)], lhsT=h2gT[:, kc, :],
                            rhs=wmlpc[:, ts(nb, 512)],
                            start=(kc == 0), stop=(kc == 7), skip_group_check=True,
                        )
                y_sb = work.tile([B, C], F32, name="y_sb")
                nc.vector.tensor_copy(out=y_sb, in_=mlp_ps)
                nc.sync.dma_start(out=y_d, in_=y_sb)

    nc.compile()
    return nc


def _prepare(x, prev_k, prev_v, pos, ln1_scale, ln1_bias, w_attn, b_attn,
             w_attn_proj, b_attn_proj, ln2_scale, ln2_bias, w_fc, b_fc,
             w_mlp_proj, b_mlp_proj):
    """Host-side sharding/layout prep. Returns (nc, in_maps, extras)."""
    f32 = np.float32
    x = np.ascontiguousarray(np.asarray(x, f32))
    prev_k = np.asarray(prev_k, f32)
    prev_v = np.asarray(prev_v, f32)
    pos = np.asarray(pos, np.int32)
    ln1_scale = np.asarray(ln1_scale, f32); ln1_bias = np.asarray(ln1_bias, f32)
    ln2_scale = np.asarray(ln2_scale, f32); ln2_bias = np.asarray(ln2_bias, f32)
    w_attn = np.asarray(w_attn, f32); b_attn = np.asarray(b_attn, f32)
    w_attn_proj = np.asarray(w_attn_proj, f32); b_attn_proj = np.asarray(b_attn_proj, f32)
    w_fc = np.asarray(w_fc, f32); b_fc = np.asarray(b_fc, f32)
    w_mlp_proj = np.asarray(w_mlp_proj, f32); b_mlp_proj = np.asarray(b_mlp_proj, f32)

    # fold LN affine params into the following matmuls (exact algebra)
    if not np.all(ln1_scale == 1.0):
        w_attn_eff = ln1_scale[:, None] * w_attn
    else:
        w_attn_eff = w_attn
    b_attn_eff = b_attn + (ln1_bias @ w_attn if np.any(ln1_bias) else 0.0)
    b_attn_eff = np.asarray(b_attn_eff, f32)
    if not np.all(ln2_scale == 1.0):
        w_fc_eff = ln2_scale[:, None] * w_fc
    else:
        w_fc_eff = w_fc
    b_fc_eff = np.asarray(b_fc + (ln2_bias @ w_fc if np.any(ln2_bias) else 0.0), f32)

    scale = f32(1.0 / math.sqrt(HD))
    kc_bf = prev_k.astype(BF)  # [B, T, H, HD]
    vc_bf = prev_v.astype(BF)

    # additive causal mask in the scores layout [128 p, B, 16 c]; t = c*128 + p
    t_idx = np.arange(T, dtype=np.int64)
    mfull = np.where(t_idx[None, :] <= pos[:, None].astype(np.int64), f32(0), f32(NEG))
    maskadd = np.ascontiguousarray(
        mfull.astype(f32).reshape(B, NCHUNK, 128).transpose(2, 0, 1)
    )  # [128, B, 16]

    flags = (bool(np.any(b_attn_eff[:C])), bool(np.any(b_fc_eff)), bool(np.any(b_attn_proj)))
    if flags not in _CACHE:
        _CACHE[flags] = _build(flags)
    nc = _CACHE[flags]

    in_maps = []
    for i in range(N_CORES):
        hh = slice(HPC * i * HD, HPC * (i + 1) * HD)  # q/k/v column range
        heads = slice(HPC * i, HPC * (i + 1))
        kdev = np.ascontiguousarray(
            kc_bf[:, :, heads, :].transpose(2, 0, 3, 1)
        ).reshape(NPAIR, HD, T)  # [(h b), d, t]
        vdev = np.ascontiguousarray(
            vc_bf[:, :, heads, :].reshape(B, NCHUNK, 128, HPC, HD).transpose(3, 0, 2, 1, 4)
        ).reshape(NPAIR, 128, NCHUNK, HD)  # [(h b), p, c, d]

        wq = w_attn_eff[:, hh] * scale
        wk = w_attn_eff[:, C + hh.start : C + hh.stop]
        wv = w_attn_eff[:, 2 * C + hh.start : 2 * C + hh.stop]
        wqkv = np.ascontiguousarray(
            np.concatenate([wq, wk, wv], axis=1).astype(f32)
            .reshape(16, 128, 3 * HPC * HD).transpose(1, 0, 2)
        )
        wproj = np.ascontiguousarray(
            w_attn_proj[hh].reshape(HPC, HD, C).transpose(1, 0, 2).astype(BF)
        )
        wfc = np.ascontiguousarray(
            w_fc_eff[:, 1024 * i : 1024 * (i + 1)]
            .reshape(16, 128, 1024).transpose(1, 0, 2).astype(BF)
        )
        wmlp = np.ascontiguousarray(
            w_mlp_proj[1024 * i : 1024 * (i + 1)]
            .reshape(8, 128, C).transpose(1, 0, 2).astype(BF)
        )
        m = {
            "x": x, "kdev": kdev, "vdev": vdev, "maskadd": maskadd,
            "wqkv": wqkv, "wproj": wproj, "wfc": wfc, "wmlp": wmlp,
        }
        if flags[0]:
            bq = (b_attn_eff[hh] * scale).reshape(HPC, HD).T  # [128, HPC]
            m["bqT"] = np.ascontiguousarray(bq.astype(f32))
        if flags[1]:
            m["bfc"] = np.ascontiguousarray(b_fc_eff[1024 * i : 1024 * (i + 1)])
        if flags[2]:
            m["bproj"] = np.ascontiguousarray(b_attn_proj)
        in_maps.append(m)

    extras = {"b_attn_eff": b_attn_eff, "b_mlp_proj": b_mlp_proj}
    return nc, in_maps, extras


def _assemble(r, extras):
    f32 = np.float32
    b_attn_eff = extras["b_attn_eff"]
    b_mlp_proj = extras["b_mlp_proj"]

    x_out = r[0]["x2"].copy()
    for i in range(N_CORES):
        x_out += r[i]["y_mlp"]
    if np.any(b_mlp_proj):
        x_out += b_mlp_proj[None, :]

    k_new = np.concatenate(
        [r[i]["knew"].reshape(B, HPC, HD) for i in range(N_CORES)], axis=1
    )
    v_new = np.concatenate(
        [r[i]["vnew"].reshape(B, HPC, HD) for i in range(N_CORES)], axis=1
    )
    kb = b_attn_eff[C : 2 * C].reshape(H, HD)
    vb = b_attn_eff[2 * C : 3 * C].reshape(H, HD)
    if np.any(kb):
        k_new = k_new + kb[None]
    if np.any(vb):
        v_new = v_new + vb[None]

    return (
        np.ascontiguousarray(x_out, dtype=f32),
        np.ascontiguousarray(k_new[:, None], dtype=f32),
        np.ascontiguousarray(v_new[:, None], dtype=f32),
    )


def kernel(**inputs):
    nc, in_maps, extras = _prepare(**inputs)
    res = run_bass_kernel_spmd(nc, in_maps, core_ids=list(range(N_CORES)))
    return _assemble(res.results, extras)


# revision 18
# speedup vs baseline: 3.2227x; 3.2227x over previous
"""Trainium2 Bass kernel for nn_Block_89172110999857 (dense transformer decode block).

Sharding: tensor-parallel over 8 NeuronCores.
  - Attention: 2 heads per core (H=16). KV cache for those heads streamed in bf16.
  - attn out-proj: row-sharded per head -> partial [B, C], AllReduce across cores.
  - MLP: Megatron col/row sharding (w_fc columns, w_mlp_proj rows); partial outputs
    summed on the host.
Numerics:
  - QKV projection in fp32 (k_new / v_new are direct outputs).
  - Attention scores / AV and the MLP in bf16 inputs with fp32 PSUM accumulation.
  - Softmax: unnormalized exp (scores are O(1); no max subtraction needed), additive
    host-built causal mask (-1e30), denominator via fused accum_out on the exp, final
    normalization folded into per-head projection rows (tensor_scalar by 1/den).
"""

import math

import numpy as np
import ml_dtypes

import concourse.bacc as bacc
import concourse.mybir as mybir
import concourse.tile as tile
from concourse.bass import ts
from concourse.bass_utils import run_bass_kernel_spmd
from concourse.masks import make_identity

B, T, H, HD = 32, 2048, 16, 128
C = H * HD  # 2048
N_CORES = 8
HPC = H // N_CORES  # 2 heads per core
NPAIR = B * HPC  # 64 (b, h) pairs per core
NCHUNK = T // 128  # 16 t-chunks
G = 4  # pairs per KV DMA group
EPS = 1e-5
NEG = -1e30

F32 = mybir.dt.float32
BF16 = mybir.dt.bfloat16
BF = ml_dtypes.bfloat16
AF = mybir.ActivationFunctionType
ALU = mybir.AluOpType

_CACHE: dict = {}


def _layernorm(nc, pool, x_sb, out_tile, eps_sb, name):
    """out_tile = (x - mean(x)) * rsqrt(var(x) + eps), rows = partitions."""
    xg = x_sb.rearrange("b (n f) -> b n f", f=512)
    stats = pool.tile([B, 4, 6], F32, name=f"{name}_stats", tag="ln_stats")
    for i in range(4):
        nc.vector.bn_stats(out=stats[:, i, :], in_=xg[:, i, :])
    mv = pool.tile([B, 2], F32, name=f"{name}_mv", tag="ln_mv")
    nc.vector.bn_aggr(out=mv, in_=stats)
    std = pool.tile([B, 1], F32, name=f"{name}_std", tag="ln_std")
    nc.scalar.activation(out=std, in_=mv[:, 1:2], func=AF.Sqrt, bias=eps_sb, scale=1.0)
    rstd = pool.tile([B, 1], F32, name=f"{name}_rstd", tag="ln_rstd")
    nc.vector.reciprocal(out=rstd, in_=std)
    nc.vector.tensor_scalar(
        out=out_tile, in0=x_sb, scalar1=mv[:, 0:1], scalar2=rstd,
        op0=ALU.subtract, op1=ALU.mult,
    )


def _transpose_rows(nc, trps, pool, src_sb, nchunks, out_dtype, identity, name):
    """src_sb [32, nchunks*128] -> dst [128, nchunks, 32] (cast to out_dtype)."""
    dst = pool.tile([128, nchunks, 32], out_dtype, name=name)
    for i in range(nchunks):
        trp = trps.tile([128, 32], src_sb.dtype, name=f"{name}_trp{i}", tag="trp")
        nc.tensor.transpose(out=trp, in_=src_sb[:, ts(i, 128)], identity=identity)
        nc.vector.tensor_copy(out=dst[:, i, :], in_=trp)
    return dst


def _build(flags):
    import os
    sim_1core = os.environ.get("KERNEL_SIM_1CORE") == "1"
    has_bq, has_bfc, has_bproj = flags
    nc = bacc.Bacc(
        "TRN2", target_bir_lowering=False, debug=False,
        num_devices=1 if sim_1core else N_CORES,
    )

    x_d = nc.dram_tensor("x", [B, C], F32, kind="ExternalInput").ap()
    kdev_d = nc.dram_tensor("kdev", [NPAIR, HD, T], BF16, kind="ExternalInput").ap()
    vdev_d = nc.dram_tensor("vdev", [NPAIR, 128, NCHUNK, HD], BF16, kind="ExternalInput").ap()
    mask_d = nc.dram_tensor("maskadd", [128, B, NCHUNK], F32, kind="ExternalInput").ap()
    wqkv_d = nc.dram_tensor("wqkv", [128, 16, 3 * HPC * HD], F32, kind="ExternalInput").ap()
    wproj_d = nc.dram_tensor("wproj", [128, HPC, C], BF16, kind="ExternalInput").ap()
    wfc_d = nc.dram_tensor("wfc", [128, 16, 1024], BF16, kind="ExternalInput").ap()
    wmlp_d = nc.dram_tensor("wmlp", [128, 8, C], BF16, kind="ExternalInput").ap()
    if has_bq:
        bq_d = nc.dram_tensor("bqT", [HD, HPC], F32, kind="ExternalInput").ap()
    if has_bfc:
        bfc_d = nc.dram_tensor("bfc", [1024], F32, kind="ExternalInput").ap()
    if has_bproj:
        bproj_d = nc.dram_tensor("bproj", [C], F32, kind="ExternalInput").ap()

    y_d = nc.dram_tensor("y_mlp", [B, C], F32, kind="ExternalOutput").ap()
    x2_d = nc.dram_tensor("x2", [B, C], F32, kind="ExternalOutput").ap()
    knew_d = nc.dram_tensor("knew", [B, HPC * HD], F32, kind="ExternalOutput").ap()
    vnew_d = nc.dram_tensor("vnew", [B, HPC * HD], F32, kind="ExternalOutput").ap()

    QN = HPC * HD  # 256: per-core q/k/v width

    with tile.TileContext(nc) as tc:
        from contextlib import ExitStack

        with ExitStack() as ctx:
            const = ctx.enter_context(tc.tile_pool(name="const", bufs=1))
            work = ctx.enter_context(tc.tile_pool(name="work", bufs=1))
            wstream = ctx.enter_context(tc.tile_pool(name="wstream", bufs=2))
            wpool = ctx.enter_context(tc.tile_pool(name="wpool", bufs=1))
            kvp = ctx.enter_context(tc.tile_pool(name="kvp", bufs=2))
            dram = ctx.enter_context(tc.tile_pool(name="dram", bufs=1, space="DRAM"))

            # ---- constants ----
            id_f = const.tile([32, 32], F32)
            make_identity(nc, id_f)
            id_b = const.tile([32, 32], BF16)
            make_identity(nc, id_b)
            eps_sb = const.tile([B, 1], F32)
            nc.vector.memset(eps_sb, EPS)
            ones_sb = const.tile([128, 1], F32)
            nc.vector.memset(ones_sb, 1.0)

            mask_sb = const.tile([128, B, NCHUNK], F32)
            nc.sync.dma_start(out=mask_sb, in_=mask_d)
            wproj_sb = wpool.tile([128, HPC, C], BF16)
            nc.sync.dma_start(out=wproj_sb, in_=wproj_d)

            x_sb = const.tile([B, C], F32)
            nc.sync.dma_start(out=x_sb, in_=x_d)

            # ---- LN1 + QKV (fp32) ----
            h_sb = work.tile([B, C], F32, name="h_sb")
            _layernorm(nc, work, x_sb, h_sb, eps_sb, "ln1")

            qT = work.tile([128, NPAIR], BF16, name="qT")
            with tc.tile_pool(name="trps1", bufs=2, space="PSUM") as trps, \
                 tc.tile_pool(name="qkvps", bufs=1, space="PSUM") as qkvps:
                hT = _transpose_rows(nc, trps, work, h_sb, 16, F32, id_f, "hT")

                q_ps = qkvps.tile([B, QN], F32, name="q_ps")
                k_ps = qkvps.tile([B, QN], F32, name="k_ps")
                v_ps = qkvps.tile([B, QN], F32, name="v_ps")
                for kc in range(16):
                    wqc = wstream.tile([128, 3 * QN], F32, name="wqc", tag="wqc")
                    nc.sync.dma_start(out=wqc, in_=wqkv_d[:, kc, :])
                    for blk, (n0, n1) in zip(
                        (q_ps, k_ps, v_ps), ((0, QN), (QN, 2 * QN), (2 * QN, 3 * QN))
                    ):
                        nc.tensor.matmul(
                            out=blk, lhsT=hT[:, kc, :], rhs=wqc[:, n0:n1],
                            start=(kc == 0), stop=(kc == 15),
                        )

                k_sb = work.tile([B, QN], F32, name="k_sb")
                nc.vector.tensor_copy(out=k_sb, in_=k_ps)
                nc.sync.dma_start(out=knew_d, in_=k_sb)
                v_sb = work.tile([B, QN], F32, name="v_sb")
                nc.vector.tensor_copy(out=v_sb, in_=v_ps)
                nc.sync.dma_start(out=vnew_d, in_=v_sb)

                # qT [128 d, (h b)] bf16 (1/sqrt(HD) folded into wq on host)
                q_sb = work.tile([B, QN], F32, name="q_sb")
                nc.vector.tensor_copy(out=q_sb, in_=q_ps)
                if has_bq:
                    bq_sb = const.tile([HD, HPC], F32)
                    nc.sync.dma_start(out=bq_sb, in_=bq_d)
                for h in range(HPC):
                    qtp = trps.tile([128, 32], F32, name=f"qtp{h}", tag="trp")
                    nc.tensor.transpose(out=qtp, in_=q_sb[:, ts(h, HD)], identity=id_f)
                    if has_bq:
                        nc.vector.tensor_scalar(
                            out=qT[:, ts(h, 32)], in0=qtp, scalar1=bq_sb[:, h : h + 1],
                            scalar2=None, op0=ALU.add,
                        )
                    else:
                        nc.vector.tensor_copy(out=qT[:, ts(h, 32)], in_=qtp)

            # ---- attention over 64 (b, h) pairs ----
            den_all = work.tile([128, NPAIR], F32, name="den_all")
            den_inv = work.tile([B, HPC], F32, name="den_inv")
            attnT_sb = work.tile([128, NPAIR], BF16, name="attnT_sb")

            with tc.tile_pool(name="scps", bufs=2, space="PSUM") as scps, \
                 tc.tile_pool(name="atps", bufs=1, space="PSUM") as atps:
                attnT_ps = [
                    atps.tile([128, B], F32, name=f"attnT{h}") for h in range(HPC)
                ]

                for j0 in range(0, NPAIR, G):
                    k_t = kvp.tile([128, G, T], BF16, name="k_t")
                    nc.sync.dma_start(
                        out=k_t, in_=kdev_d[j0 : j0 + G].rearrange("j p t -> p j t")
                    )
                    v_t = kvp.tile([128, G, NCHUNK, HD], BF16, name="v_t")
                    nc.sync.dma_start(
                        out=v_t, in_=vdev_d[j0 : j0 + G].rearrange("j p c d -> p j c d")
                    )
                    # software-pipelined trace order within the group: all
                    # scores blocks first, then softmax, then all AV blocks —
                    # keeps the PE stream free of per-pair DVE/ACT stalls.
                    scs = []
                    for jj in range(G):
                        j = j0 + jj
                        sc = scps.tile(
                            [128, NCHUNK], F32, name="sc", tag="sc", bufs=G + 1
                        )
                        scs.append(sc)
                        for c in range(NCHUNK):
                            nc.tensor.matmul(
                                out=sc[:, c : c + 1], lhsT=k_t[:, jj, ts(c, 128)],
                                rhs=qT[:, j : j + 1], start=True, stop=True,
                                skip_group_check=True,
                            )
                    prs = []
                    for jj in range(G):
                        j = j0 + jj
                        b = j % B
                        sm = work.tile(
                            [128, NCHUNK], F32, name="sm", tag="sm", bufs=G + 1
                        )
                        nc.vector.tensor_add(out=sm, in0=scs[jj], in1=mask_sb[:, b, :])
                        pr = work.tile(
                            [128, NCHUNK], BF16, name="pr", tag="pr", bufs=G + 1
                        )
                        prs.append(pr)
                        nc.scalar.activation(
                            out=pr, in_=sm, func=AF.Exp, accum_out=den_all[:, j : j + 1]
                        )
                    for jj in range(G):
                        j = j0 + jj
                        b, h = j % B, j // B
                        for c in range(NCHUNK):
                            nc.tensor.matmul(
                                out=attnT_ps[h][:, b : b + 1], lhsT=v_t[:, jj, c, :],
                                rhs=prs[jj][:, c : c + 1],
                                start=(c == 0), stop=(c == NCHUNK - 1),
                                skip_group_check=True,
                            )

                # denominators -> 1/den per head [32, 1]
                for h in range(HPC):
                    den_ps = scps.tile([B, 1], F32, name=f"den_ps{h}", tag="den_ps", bufs=1)
                    nc.tensor.matmul(
                        out=den_ps, lhsT=den_all[:, ts(h, 32)], rhs=ones_sb,
                        start=True, stop=True,
                    )
                    nc.vector.reciprocal(out=den_inv[:, h : h + 1], in_=den_ps)

                for h in range(HPC):
                    nc.vector.tensor_copy(out=attnT_sb[:, ts(h, 32)], in_=attnT_ps[h])

            # per-head out-proj (contraction d=128), normalize rows by 1/den
            attn_par = work.tile([B, C], F32, name="attn_par")
            with tc.tile_pool(name="projps", bufs=1, space="PSUM") as projps:
                proj_ps = [
                    projps.tile([B, C], F32, name=f"proj_ps{h}") for h in range(HPC)
                ]
                for h in range(HPC):
                    for nb in range(C // 512):
                        nc.tensor.matmul(
                            out=proj_ps[h][:, ts(nb, 512)], lhsT=attnT_sb[:, ts(h, 32)],
                            rhs=wproj_sb[:, h, ts(nb, 512)], start=True, stop=True,
                        )
                pn0 = work.tile([B, C], F32, name="pn0")
                nc.vector.tensor_scalar_mul(
                    out=pn0, in0=proj_ps[0], scalar1=den_inv[:, 0:1]
                )
                pn1 = work.tile([B, C], F32, name="pn1")
                nc.vector.tensor_scalar_mul(
                    out=pn1, in0=proj_ps[1], scalar1=den_inv[:, 1:2]
                )
                nc.vector.tensor_add(out=attn_par, in0=pn0, in1=pn1)

            # ---- AllReduce of the attention partial ----
            ar_in = dram.tile([B, C], F32, name="ar_in")
            ar_out = dram.tile([B, C], F32, name="ar_out", addr_space="Shared")
            nc.sync.dma_start(out=ar_in, in_=attn_par)
            if sim_1core:
                nc.sync.dma_start(out=ar_out, in_=ar_in)
            else:
                nc.gpsimd.collective_compute(
                    "AllReduce", ALU.add,
                    replica_groups=[list(range(N_CORES))],
                    ins=[ar_in.opt()], outs=[ar_out.opt()],
                )
            ar_sb = work.tile([B, C], F32, name="ar_sb")
            nc.sync.dma_start(out=ar_sb, in_=ar_out)

            x2_sb = work.tile([B, C], F32, name="x2_sb")
            if has_bproj:
                bproj_sb = const.tile([B, C], F32)
                import concourse.bass as bass_mod
                bproj_bcast = bass_mod.AP(
                    tensor=bproj_d.tensor, offset=0, ap=[[0, B], [1, C]]
                )
                nc.sync.dma_start(out=bproj_sb, in_=bproj_bcast)
                tmpb = work.tile([B, C], F32, name="tmpb")
                nc.vector.tensor_add(out=tmpb, in0=ar_sb, in1=bproj_sb)
                nc.vector.tensor_add(out=x2_sb, in0=x_sb, in1=tmpb)
            else:
                nc.vector.tensor_add(out=x2_sb, in0=x_sb, in1=ar_sb)
            nc.sync.dma_start(out=x2_d, in_=x2_sb)

            # ---- LN2 + MLP (bf16) ----
            h2_bf = work.tile([B, C], BF16, name="h2_bf")
            _layernorm(nc, work, x2_sb, h2_bf, eps_sb, "ln2")

            with tc.tile_pool(name="trps2", bufs=2, space="PSUM") as trps2, \
                 tc.tile_pool(name="mlpps", bufs=1, space="PSUM") as mlpps:
                h2T = _transpose_rows(nc, trps2, work, h2_bf, 16, BF16, id_b, "h2T")

                fc_ps = mlpps.tile([B, 1024], F32, name="fc_ps")
                for kc in range(16):
                    wfcc = wstream.tile([128, 1024], BF16, name="wfcc", tag="wfcc")
                    nc.sync.dma_start(out=wfcc, in_=wfc_d[:, kc, :])
                    for nb in range(2):
                        nc.tensor.matmul(
                            out=fc_ps[:, ts(nb, 512)], lhsT=h2T[:, kc, :],
                            rhs=wfcc[:, ts(nb, 512)],
                            start=(kc == 0), stop=(kc == 15), skip_group_check=True,
                        )
                h2g = work.tile([B, 1024], BF16, name="h2g")
                if has_bfc:
                    bfc_sb = const.tile([B, 1024], F32)
                    import concourse.bass as bass_mod
                    bfc_bcast = bass_mod.AP(
                        tensor=bfc_d.tensor, offset=0, ap=[[0, B], [1, 1024]]
                    )
                    nc.sync.dma_start(out=bfc_sb, in_=bfc_bcast)
                    fcb = work.tile([B, 1024], F32, name="fcb")
                    nc.vector.tensor_add(out=fcb, in0=fc_ps, in1=bfc_sb)
                    nc.scalar.activation(out=h2g, in_=fcb, func=AF.Gelu_apprx_tanh)
                else:
                    nc.scalar.activation(out=h2g, in_=fc_ps, func=AF.Gelu_apprx_tanh)

                h2gT = _transpose_rows(nc, trps2, work, h2g, 8, BF16, id_b, "h2gT")
                mlp_ps = mlpps.tile([B, C], F32, name="mlp_ps")
                for kc in range(8):
                    wmlpc = wstream.tile([128, C], BF16, name="wmlpc", tag="wmlpc")
                    nc.sync.dma_start(out=wmlpc, in_=wmlp_d[:, kc, :])
                    for nb in range(4):
                        nc.tensor.matmul(
                            out=mlp_ps[:, ts(nb, 512)], lhsT=h2gT[:, kc, :],
                            rhs=wmlpc[:, ts(nb, 512)],
                            start=(kc == 0), stop=(kc == 7), skip_group_check=True,
                        )
                y_sb = work.tile([B, C], F32, name="y_sb")
                nc.vector.tensor_copy(out=y_sb, in_=mlp_ps)
                nc.sync.dma_start(out=y_d, in_=y_sb)

    nc.compile()
    return nc


def _prepare(x, prev_k, prev_v, pos, ln1_scale, ln1_bias, w_attn, b_attn,
             w_attn_proj, b_attn_proj, ln2_scale, ln2_bias, w_fc, b_fc,
             w_mlp_proj, b_mlp_proj):
    """Host-side sharding/layout prep. Returns (nc, in_maps, extras)."""
    f32 = np.float32
    x = np.ascontiguousarray(np.asarray(x, f32))
    prev_k = np.asarray(prev_k, f32)
    prev_v = np.asarray(prev_v, f32)
    pos = np.asarray(pos, np.int32)
    ln1_scale = np.asarray(ln1_scale, f32); ln1_bias = np.asarray(ln1_bias, f32)
    ln2_scale = np.asarray(ln2_scale, f32); ln2_bias = np.asarray(ln2_bias, f32)
    w_attn = np.asarray(w_attn, f32); b_attn = np.asarray(b_attn, f32)
    w_attn_proj = np.asarray(w_attn_proj, f32); b_attn_proj = np.asarray(b_attn_proj, f32)
    w_fc = np.asarray(w_fc, f32); b_fc = np.asarray(b_fc, f32)
    w_mlp_proj = np.asarray(w_mlp_proj, f32); b_mlp_proj = np.asarray(b_mlp_proj, f32)

    # fold LN affine params into the following matmuls (exact algebra)
    if not np.all(ln1_scale == 1.0):
        w_attn_eff = ln1_scale[:, None] * w_attn
    else:
        w_attn_eff = w_attn
    b_attn_eff = b_attn + (ln1_bias @ w_attn if np.any(ln1_bias) else 0.0)
    b_attn_eff = np.asarray(b_attn_eff, f32)
    if not np.all(ln2_scale == 1.0):
        w_fc_eff = ln2_scale[:, None] * w_fc
    else:
        w_fc_eff = w_fc
    b_fc_eff = np.asarray(b_fc + (ln2_bias @ w_fc if np.any(ln2_bias) else 0.0), f32)

    scale = f32(1.0 / math.sqrt(HD))
    kc_bf = prev_k.astype(BF)  # [B, T, H, HD]
    vc_bf = prev_v.astype(BF)

    # additive causal mask in the scores layout [128 p, B, 16 c]; t = c*128 + p
    t_idx = np.arange(T, dtype=np.int64)
    mfull = np.where(t_idx[None, :] <= pos[:, None].astype(np.int64), f32(0), f32(NEG))
    maskadd = np.ascontiguousarray(
        mfull.astype(f32).reshape(B, NCHUNK, 128).transpose(2, 0, 1)
    )  # [128, B, 16]

    import os as _os
    flags = (bool(np.any(b_attn_eff[:C])), bool(np.any(b_fc_eff)), bool(np.any(b_attn_proj)))
    key = (flags, _os.environ.get("KERNEL_SIM_1CORE") == "1")
    if key not in _CACHE:
        _CACHE[key] = _build(flags)
    nc = _CACHE[key]

    in_maps = []
    for i in range(N_CORES):
        hh = slice(HPC * i * HD, HPC * (i + 1) * HD)  # q/k/v column range
        heads = slice(HPC * i, HPC * (i + 1))
        kdev = np.ascontiguousarray(
            kc_bf[:, :, heads, :].transpose(2, 0, 3, 1)
        ).reshape(NPAIR, HD, T)  # [(h b), d, t]
        vdev = np.ascontiguousarray(
            vc_bf[:, :, heads, :].reshape(B, NCHUNK, 128, HPC, HD).transpose(3, 0, 2, 1, 4)
        ).reshape(NPAIR, 128, NCHUNK, HD)  # [(h b), p, c, d]

        wq = w_attn_eff[:, hh] * scale
        wk = w_attn_eff[:, C + hh.start : C + hh.stop]
        wv = w_attn_eff[:, 2 * C + hh.start : 2 * C + hh.stop]
        wqkv = np.ascontiguousarray(
            np.concatenate([wq, wk, wv], axis=1).astype(f32)
            .reshape(16, 128, 3 * HPC * HD).transpose(1, 0, 2)
        )
        wproj = np.ascontiguousarray(
            w_attn_proj[hh].reshape(HPC, HD, C).transpose(1, 0, 2).astype(BF)
        )
        wfc = np.ascontiguousarray(
            w_fc_eff[:, 1024 * i : 1024 * (i + 1)]
            .reshape(16, 128, 1024).transpose(1, 0, 2).astype(BF)
        )
        wmlp = np.ascontiguousarray(
            w_mlp_proj[1024 * i : 1024 * (i + 1)]
            .reshape(8, 128, C).transpose(1, 0, 2).astype(BF)
        )
        m = {
            "x": x, "kdev": kdev, "vdev": vdev, "maskadd": maskadd,
            "wqkv": wqkv, "wproj": wproj, "wfc": wfc, "wmlp": wmlp,
        }
        if flags[0]:
            bq = (b_attn_eff[hh] * scale).reshape(HPC, HD).T  # [128, HPC]
            m["bqT"] = np.ascontiguousarray(bq.astype(f32))
        if flags[1]:
            m["bfc"] = np.ascontiguousarray(b_fc_eff[1024 * i : 1024 * (i + 1)])
        if flags[2]:
            m["bproj"] = np.ascontiguousarray(b_attn_proj)
        in_maps.append(m)

    extras = {"b_attn_eff": b_attn_eff, "b_mlp_proj": b_mlp_proj}
    return nc, in_maps, extras


def _assemble(r, extras):
    f32 = np.float32
    b_attn_eff = extras["b_attn_eff"]
    b_mlp_proj = extras["b_mlp_proj"]

    x_out = r[0]["x2"].copy()
    for i in range(N_CORES):
        x_out += r[i]["y_mlp"]
    if np.any(b_mlp_proj):
        x_out += b_mlp_proj[None, :]

    k_new = np.concatenate(
        [r[i]["knew"].reshape(B, HPC, HD) for i in range(N_CORES)], axis=1
    )
    v_new = np.concatenate(
        [r[i]["vnew"].reshape(B, HPC, HD) for i in range(N_CORES)], axis=1
    )
    kb = b_attn_eff[C : 2 * C].reshape(H, HD)
    vb = b_attn_eff[2 * C : 3 * C].reshape(H, HD)
    if np.any(kb):
        k_new = k_new + kb[None]
    if np.any(vb):
        v_new = v_new + vb[None]

    return (
        np.ascontiguousarray(x_out, dtype=f32),
        np.ascontiguousarray(k_new[:, None], dtype=f32),
        np.ascontiguousarray(v_new[:, None], dtype=f32),
    )


def kernel(**inputs):
    nc, in_maps, extras = _prepare(**inputs)
    res = run_bass_kernel_spmd(nc, in_maps, core_ids=list(range(N_CORES)))
    return _assemble(res.results, extras)
